# revision 69
# baseline (speedup 1.0000x reference)
"""DiffusionGraphConv Trainium2 kernel (8-core SPMD, data-parallel over batch).

Math refactoring (halves the big-matmul FLOPs vs the reference order):
  reference: out[b,n,o] = sum_{f,m} mats_m[n,f,b] * W[f*5+m, o]
  with mats = [x0, s0 x0, 2 s0^2 x0 - x0, s1 x0, 2 s1^2 x0 - x0].
  Projection (width F=128 -> O=64) commutes with the node-space diffusion, so:
    u_m = proj(x0, W_m)                       # [N, O, B] each, cheap
    out = (u0 - u2 - u4) + s0 (u1 + 2 s0 u2) + s1 (u3 + 2 s1 u4)
  Device computes: v0 = proj(x0, W0-W2-W4), and with pre-scaled 2*W2 / 2*W4:
    c0 = u1 + s0 @ u2s ; c1 = u3 + s1 @ u4s ; out = v0 + s0 @ c0 + s1 @ c1

Schedule (v9, tuned against TimelineSim; modeled ~86.1 us/core vs 119.5 us
for the original pipeline):
  - Phase 1 projects only the four u-slots (256 cols -> 2-bank quad psum,
    4 batches each), each quad drained by ONE du copy. Single-consumer
    drains matter: a double-drain on one psum tile couples the slot ring
    through two engines' queues and empirically halves phase1 throughput.
    v0 never materializes: it is injected into the final psum groups as
    bf16 matmuls with host-prescaled 4096*Wv0 (accumulating coherently
    with the DoubleRow terms), which removes ~10us of psum-drain work
    from phase1's copy-bound window and improves accuracy (v0 stays f32
    until the output cast).
  - Drain copies go ONLY to DVE/Act (gpsimd cannot access PSUM on real
    hw — walrus rejects it). Act is 20% faster at psum reads, so it takes
    9/16 of the du copies (~17.9 us of drain per engine paces phase1).
  - Supports live RESIDENT in SBUF (DMA'd once, 8 MB total) instead of
    being streamed twice; the diffusion phases have zero DMA dependence.
    Diffusion runs PE-bound at ~100% occupancy (54.6 us floor: fp8
    DoubleRow, K=256/matmul, full 512-row psum-bank streams).
  - V tiles are a small stream pool; output DMA'd as bf16 (host upcasts);
    the last tile drains as two halves so the tail DMA is half-size.
  - The first x0 DMA is a quarter-tile so phase1's first matmul starts
    ~1 us earlier; early x0 chunks are single-tile to match the matmul-
    rate warmup. ALL s0 strip chunks dispatch before s1's: hop1 (which
    starts ~24us, right after phase1) consumes s0 just as the shared
    serial DMA queue delivers it, and hop2 needs s1 ~14us later.
  - The final phase's last tile runs as two half-width psum groups so the
    tail's last drain+DMA chain is half-length. The serial DMA queue
    (x0 11.7us + strips 23.3us at shared bandwidth) and the two psum-
    drain engines, not PE scheduling, bound the front of the kernel.

Per-core work: 4 matmuls [2048,2048]@[2048,512] (fp8 DoubleRow, fp32 PSUM)
+ 128 projection matmuls. Layouts (host-prepared, all "SBUF images"):
  x0t  [128 f, 16t*8b*128j] bf16: x0t[f, (t*8+b)*128+j] = cat(inputs,state)[b, t*128+j, f]
  wcat [128 f, 5*64]        bf16: [4096*(W0-W2-W4) | 16*W1 | 2*W2/16 | 16*W3 | 2*W4/16]
  s*t  [16 t, 128 p, 2048]  fp8: s*t[t, p, kt*128+j] = SCALE * s[t*128+j, kt*128+p]
       (strip t = transposed rows of s for output-node tile t, k-major)
  out  [2048 n, 8b*64o]     bf16

Env quirks handled here: walrus accepts <=1 sync-wait per instruction
(_legalize_waits hoists extras onto EventSemaphore carriers; simulators need
legalize=False); repeat=N re-runs the idempotent pipeline for wall-clock
differencing since this axon terminal has no NTFF profiling.
"""

import sys

if "/opt/trn_rl_repo" not in sys.path:
    sys.path.insert(0, "/opt/trn_rl_repo")

import numpy as np
import ml_dtypes

import concourse.bass as bass
import concourse.mybir as mybir
from concourse.tile import TileContext
from concourse.bass_utils import run_bass_kernel_spmd

BF16 = mybir.dt.bfloat16
FP8 = mybir.dt.float8e4
NPFP8 = ml_dtypes.float8_e4m3
SCALE = 256.0
F32 = mybir.dt.float32
NPBF16 = ml_dtypes.bfloat16

N = 2048          # graph nodes
F = 128           # input_size (64 input + 64 hidden)
B = 64            # global batch
NCORES = 8
BS = B // NCORES  # 8 batches per core
O = 64            # output features
NT = N // 128     # 16 node tiles
M5 = 5            # diffusion matrices


def _legalize_waits(nc, max_waits=1):
    """Walrus in this env encodes at most one sync-wait per instruction.

    Tile's sem assignment can emit 2-3 waits on one instruction; hoist the
    excess onto standalone EventSemaphore carriers (same engine, inserted
    just before), which the sequencer executes in order — semantics are
    identical, encoding is legal."""
    f = nc.m.functions[0]
    for blk in f.blocks:
        new_insts = []
        changed = False
        for inst in blk.instructions:
            si = inst.sync_info
            waits = list(si.on_wait) if si is not None else []
            if len(waits) > max_waits:
                for i, w in enumerate(waits[:-max_waits]):
                    ev = mybir.InstEventSemaphore(
                        name=f"{inst.name}-wsplit{i}",
                        engine=inst.engine,
                        ins=[],
                        outs=[],
                        sync_info=mybir.SyncInfo(on_wait=[w], on_update=[]),
                    )
                    new_insts.append(ev)
                inst.sync_info = mybir.SyncInfo(
                    on_wait=waits[-max_waits:], on_update=list(si.on_update)
                )
                changed = True
            new_insts.append(inst)
        if changed:
            blk.instructions = new_insts
    return nc


def build_bass(n=N, bs=BS, o=O, legalize=True, n_hops=4, repeat=1):
    """Build the per-core SPMD Bass program."""
    nt = n // 128
    nc = bass.Bass()
    x0t = nc.dram_tensor("x0t", [F, bs * n], BF16, kind="ExternalInput")
    wcat = nc.dram_tensor("wcat", [F, M5 * o], BF16, kind="ExternalInput")
    s0t = nc.dram_tensor("s0t", [nt, 128, n], FP8, kind="ExternalInput")
    s1t = nc.dram_tensor("s1t", [nt, 128, n], FP8, kind="ExternalInput")
    out = nc.dram_tensor("out", [n, bs * o], BF16, kind="ExternalOutput")

    obs = bs * o        # 512: width of diffusion operands
    with TileContext(nc) as tc:
        with (
            tc.tile_pool(name="persist", bufs=1) as persist,
            tc.tile_pool(name="vout", bufs=4) as vout,
            tc.tile_pool(name="pp", bufs=4, space="PSUM") as pp,
        ):
            # x0t is t-major on host: free index = t*bs*128 + b*128 + j, so
            # each node-tile's stationary slices arrive in one chunk DMA.
            # The first chunk (tiles 0-1) and the weights dispatch FIRST so
            # phase1 can start while the rest streams in.
            w_sb = persist.tile([F, M5 * o], BF16, name="w_sb")
            x0_sb = persist.tile([F, bs * n], BF16, name="x0_sb")
            nc.sync.dma_start(
                out=x0_sb[:, 0:bs * 64], in_=x0t[:, 0:bs * 64])
            nc.sync.dma_start(out=w_sb[:, :], in_=wcat[:, :])
            nc.sync.dma_start(
                out=x0_sb[:, bs * 64:bs * 256], in_=x0t[:, bs * 64:bs * 256])
            # single-tile chunks while phase1 consumes at matmul rate, then
            # 2-tile chunks once the psum-drain pace takes over
            for t1 in range(2, 6):
                nc.sync.dma_start(
                    out=x0_sb[:, t1 * bs * 128:(t1 + 1) * bs * 128],
                    in_=x0t[:, t1 * bs * 128:(t1 + 1) * bs * 128],
                )
            for c in range(3, nt // 2):
                nc.sync.dma_start(
                    out=x0_sb[:, c * bs * 256:(c + 1) * bs * 256],
                    in_=x0t[:, c * bs * 256:(c + 1) * bs * 256],
                )
            # Resident support strips: DMA'd once, reused by hops AND final.
            s0_sb = persist.tile([128, nt * n], FP8, name="s0_sb")
            s1_sb = persist.tile([128, nt * n], FP8, name="s1_sb")
            # all s0 chunks first: hop1 consumes them from ~24us, while
            # hop2 (s1) starts ~14us later — ordering matches consumption
            for sb_, st_ in ((s0_sb, s0t), (s1_sb, s1t)):
                for c in range(nt // 4):
                    nc.sync.dma_start(
                        out=sb_[:, c * 4 * n:(c + 1) * 4 * n].rearrange(
                            "p (t k) -> p t k", t=4),
                        in_=st_[c * 4:(c + 1) * 4].rearrange("t p k -> p t k"),
                    )

            def strip(sb, t):
                return sb[:, t * n:(t + 1) * n]

            # U[tp]: [128, 4mi*2kt*bs*o] fp8 — the four projection slots for a
            # k-tile pair: mi 1=u1->c0, 2=2*u2, 3=u3->c1, 4=2*u4 (v0 lives
            # separately in bf16 V0 tiles for precision).
            U = [
                persist.tile([128, 4 * 2 * obs], FP8, name=f"u{tp}", tag=f"u{tp}")
                for tp in range(nt // 2)
            ]
            def upair(tp, mi):
                """[128, 2, obs] DoubleRow moving view: k-tile pair of slot mi."""
                return U[tp].rearrange("p (mi4 kt2 c) -> p mi4 kt2 c", mi4=4, kt2=2)[
                    :, mi - 1, :, :
                ]

            def uslot(t, mi):
                """[128, obs] contiguous view of slot mi for node-tile t."""
                base = (mi - 1) * 2 * obs + (t % 2) * obs
                return U[t // 2][:, base:base + obs]

            # ---- Phase 1: projections. The u-slot matmuls (256 cols) and the
            # v0 matmuls (64 cols) write SEPARATE psum banks: each 4-batch
            # quad psum drains with ONE du copy (DVE/Act only — gpsimd
            # cannot read PSUM on real hw), while v0 accumulates a whole
            # tile's 8 batches in one vp bank drained by ONE copy per tile —
            # keeping the drain loop single-consumer (a du+v0 double-drain
            # couples the slot ring and halves throughput).
            copy_engines = [
                lambda out, in_: nc.vector.tensor_copy(out=out, in_=in_),
                lambda out, in_: nc.scalar.copy(out=out, in_=in_),
            ]
            pair_idx = [0]

            def phase1(t):
                for bp in range(bs // 4):
                    ps = pp.tile([128, 4 * 256], F32, name="ps_proj", tag="pp")
                    for h in range(4):
                        b = bp * 4 + h
                        lhsT = x0_sb[:, (t * bs + b) * 128:(t * bs + b + 1) * 128]
                        nc.tensor.matmul(
                            ps[:, h * 256:(h + 1) * 256],
                            lhsT=lhsT,
                            rhs=w_sb[:, o:M5 * o],
                            start=True,
                            stop=True,
                        )
                    b0 = bp * 4
                    su = ps.rearrange("p (h mi4 oo) -> p h mi4 oo", h=4, oo=o)
                    du = U[t // 2].rearrange(
                        "p (mi4 kt2 bb oo) -> p mi4 kt2 bb oo", mi4=4, kt2=2, bb=bs
                    )[:, :, t % 2, b0:b0 + 4, :].rearrange(
                        "p mi4 bb oo -> p bb mi4 oo"
                    )
                    i = pair_idx[0]
                    pair_idx[0] += 1
                    # Act (1.2 GHz) drains psum ~20% faster than DVE: give it
                    # 9 of every 16 du copies; v0 copies alternate.
                    dve_du = (i % 16) in (0, 2, 5, 7, 9, 11, 14)
                    copy_engines[0 if dve_du else 1](du, su)

            # ---- Phases 2-5: diffusion hops (all operands resident).
            #   hop(s_sb, src_slot, dst):  for each node-tile t:
            #     acc = sum_kt strip[t,kt].T @ U[kt][src_slot]   (= (s @ u)[t-tile])
            def hop(s_sb, src, dst_slot):
                for t in range(nt):
                    st = strip(s_sb, t)
                    ps = pp.tile([128, obs], F32, name="ps_acc", tag="pp")
                    for ktp in range(nt // 2):
                        nc.tensor.matmul(
                            ps[:, :],
                            lhsT=st[:, ktp * 256:(ktp + 1) * 256].rearrange(
                                "p (kt2 j) -> p kt2 j", kt2=2),
                            rhs=upair(ktp, src),
                            start=(ktp == 0),
                            stop=(ktp == nt // 2 - 1),
                            perf_mode=mybir.MatmulPerfMode.DoubleRow,
                        )
                    # psum = (256*s0)@(2u2/16) = 16*(2 s0 u2); slot1 = 16*u1
                    # -> plain add keeps c0 at 16x scale (fp8-safe)
                    d = uslot(t, dst_slot)
                    nc.vector.tensor_add(d, d, ps[:, :])

            # Final phase: V = v0 + s0@c0 + s1@c1 with a single 32-matmul psum
            # group per output tile, then one fused scale+add, then DMA out.
            def final_merged():
                for t in range(nt):
                    # last tile: two independent half-width psum groups, so
                    # the first half's drain+DMA overlaps the second half's
                    # matmuls and the kernel's very last DMA is half-size
                    col_groups = ([(0, obs)] if t < nt - 1
                                  else [(0, obs // 2), (obs // 2, obs)])
                    for lo, hi in col_groups:
                        ps = pp.tile([128, hi - lo], F32, name="ps_acc", tag="pp")
                        for g, (sb, sl) in enumerate([(s0_sb, 1), (s1_sb, 3)]):
                            st = strip(sb, t)
                            for ktp in range(nt // 2):
                                nc.tensor.matmul(
                                    ps[:, :],
                                    lhsT=st[:, ktp * 256:(ktp + 1) * 256].rearrange(
                                        "p (kt2 j) -> p kt2 j", kt2=2),
                                    rhs=upair(ktp, sl)[:, :, lo:hi],
                                    start=(g == 0 and ktp == 0),
                                    stop=(g == 1 and ktp == nt // 2 - 1),
                                    perf_mode=mybir.MatmulPerfMode.DoubleRow,
                                    skip_group_check=True,
                                )
                            if g == 0:
                                # inject v0 = x0 @ (4096*Wv0) straight into the
                                # accumulation group (bf16, one matmul per batch)
                                for b in range(bs):
                                    blo, bhi = b * o, (b + 1) * o
                                    if bhi <= lo or blo >= hi:
                                        continue
                                    nc.tensor.matmul(
                                        ps[:, blo - lo:bhi - lo],
                                        lhsT=x0_sb[:, (t * bs + b) * 128:
                                                   (t * bs + b + 1) * 128],
                                        rhs=w_sb[:, 0:o],
                                        start=False,
                                        stop=False,
                                        skip_group_check=True,
                                    )
                        v = vout.tile([128, hi - lo], BF16, name="v", tag="v")
                        nc.scalar.mul(v[:, :], ps[:, :], 1.0 / (SCALE * 16.0))
                        nc.sync.dma_start(
                            out=out[t * 128:(t + 1) * 128, lo:hi], in_=v[:, :]
                        )

            # repeat>1 re-runs the whole idempotent pipeline (each round
            # rebuilds U from x0) — used only to measure per-round device
            # time via wall-clock differencing.
            for _rep in range(repeat):
                for t in range(nt):
                    phase1(t)
                if n_hops >= 4:
                    hop(s0_sb, 2, 1)     # c0 = u1 + s0 @ (2 u2)
                    hop(s1_sb, 4, 3)     # c1 = u3 + s1 @ (2 u4)
                    final_merged()       # V = v0 + s0@c0 + s1@c1 ; dma out
                else:
                    hops = [(s0_sb, 2, 1), (s1_sb, 4, 3)]
                    for hargs in hops[:n_hops]:
                        hop(*hargs)
    return _legalize_waits(nc) if legalize else nc


_NC_CACHE = {}


def _get_nc():
    if "nc" not in _NC_CACHE:
        _NC_CACHE["nc"] = build_bass()
    return _NC_CACHE["nc"]


def make_inputs(support0, support1, inputs, state, weight):
    """Host-side layout prep -> per-core in_maps (shared replicated arrays)."""
    xs = np.concatenate(
        [
            np.asarray(inputs, np.float32).reshape(B, N, F // 2),
            np.asarray(state, np.float32).reshape(B, N, F // 2),
        ],
        axis=2,
    )  # [B, N, F]

    w = np.asarray(weight, np.float32).reshape(F, M5, O)
    # wv0 carries the diffusion psum scale (SCALE*16 = 4096) so its injected
    # matmuls accumulate coherently with the DoubleRow terms in final psums.
    wv0 = (w[:, 0] - w[:, 2] - w[:, 4]) * (256.0 * 16.0)
    wcat = np.concatenate(
        [wv0, 16.0 * w[:, 1], 2.0 * w[:, 2] / 16.0,
         16.0 * w[:, 3], 2.0 * w[:, 4] / 16.0], axis=1
    ).astype(NPBF16)  # [128, 320]; hop slots scaled so fp8 adds stay in-range

    def strip_img(s):
        # fp8 DoubleRow pair layout: [t, p, ktp*256 + kt2*128 + j]
        #   = fp8(SCALE * s[t*128+j, (ktp*2+kt2)*128 + p])
        r = (SCALE * np.asarray(s, np.float32)).astype(NPFP8)
        r = r.reshape(NT, 128, NT, 128).transpose(0, 3, 2, 1)  # [t, p, kt, j]
        return np.ascontiguousarray(r.reshape(NT, 128, N))

    s0i, s1i = strip_img(support0), strip_img(support1)

    in_maps = []
    for c in range(NCORES):
        shard = xs[c * BS:(c + 1) * BS]                # [8b, N, F]
        # t-major SBUF image: x0t[f, t*BS*128 + b*128 + j] = shard[b, t*128+j, f]
        x0t = np.ascontiguousarray(
            shard.reshape(BS, NT, 128, F).transpose(3, 1, 0, 2).reshape(F, BS * N)
        ).astype(NPBF16)
        in_maps.append({"x0t": x0t, "wcat": wcat, "s0t": s0i, "s1t": s1i})
    return in_maps


def postprocess(results, biases):
    full = np.empty((B, N, O), np.float32)
    for c, r in enumerate(results):
        full[c * BS:(c + 1) * BS] = (
            r["out"].astype(np.float32).reshape(N, BS, O).transpose(1, 0, 2)
        )
    full += np.asarray(biases, np.float32)[None, None, :]
    return full.reshape(B, N * O)


def kernel(support0, support1, inputs, state, weight, biases, output_size=None,
           **run_kwargs):
    nc = _get_nc()
    in_maps = make_inputs(support0, support1, inputs, state, weight)
    res = run_bass_kernel_spmd(nc, in_maps, core_ids=list(range(NCORES)),
                               **run_kwargs)
    out = postprocess(res.results, biases)
    if run_kwargs.get("trace"):
        return out, res
    return out
